# revision 1
# baseline (speedup 1.0000x reference)
"""Fused attention block (QKV proj + KV-cache causal attention + output proj)
for Trainium2, tensor-parallel over heads across 8 NeuronCores.

Problem shapes (hardcoded): B=2, S=1024, T_past=1024, D=2048, H=16, hd=128.
Each core owns 2 heads. Inside each core:
  - qT/kT computed in [hd, token] layout (feature-major) via PE matmuls
  - v computed directly in [token, hd] layout
  - scores computed TRANSPOSED: scoresT[t, s] so the causal mask is a
    per-partition staircase (affine_select) and exp needs no max pass
    (scores ~ N(0,1), no overflow risk)
  - softmax denominator via a ones-column appended to v (col 128 of rhs)
  - ctx normalized per-partition (token on partitions), PE-transposed, then
    output projection accumulates the per-core partial in f32
Host side: shards/transposes inputs per core, sums the 8 partial outputs,
re-assembles k/v caches.
"""

import numpy as np
import ml_dtypes

import concourse.bass as bass
import concourse.bacc as bacc
import concourse.mybir as mybir
import concourse.tile as tile
from concourse.bass import ds, ts
from concourse.bass_utils import run_bass_kernel_spmd
from concourse.masks import make_identity

BF16 = mybir.dt.bfloat16
F32 = mybir.dt.float32

D = 2048
H = 16
HD = 128
B = 2
S = 1024
TP = 1024
HPC = 2               # heads per core
TOK = B * S           # 2048 tokens, b-major
KO = D // 128         # 16 contraction tiles
NT = TOK // 128       # 16 token tiles
SCALE = 1.0 / float(np.sqrt(HD))

_NC_CACHE = {}


def build_program():
    nc = bacc.Bacc("TRN2", target_bir_lowering=False, debug=False, num_devices=8)

    xT = nc.declare_dram_parameter("xT", [D, TOK], BF16, isOutput=False)
    wqT = nc.declare_dram_parameter("wqT", [D, HPC * HD], BF16, isOutput=False)
    wkT = nc.declare_dram_parameter("wkT", [D, HPC * HD], BF16, isOutput=False)
    wvT = nc.declare_dram_parameter("wvT", [D, HPC * HD], BF16, isOutput=False)
    wo = nc.declare_dram_parameter("wo", [HPC * HD, D], BF16, isOutput=False)
    pkT = nc.declare_dram_parameter("pkT", [B, HPC, HD, TP], BF16, isOutput=False)
    pve = nc.declare_dram_parameter("pve", [B, HPC, TP, 130], BF16, isOutput=False)

    kT_out = nc.declare_dram_parameter("kT_out", [HPC, HD, TOK], F32, isOutput=True)
    v_out = nc.declare_dram_parameter("v_out", [HPC, TOK, HD], F32, isOutput=True)
    partial = nc.declare_dram_parameter("partial", [TOK, D], F32, isOutput=True)

    xT_r = xT.rearrange("(ko p) t -> p ko t", p=128)
    wq_r = wqT.rearrange("(ko p) m -> p ko m", p=128)
    wk_r = wkT.rearrange("(ko p) m -> p ko m", p=128)
    wv_r = wvT.rearrange("(ko p) m -> p ko m", p=128)
    wo_r = wo.rearrange("(ft p) n -> p ft n", p=128)

    with tile.TileContext(nc) as tc:
        with (
            tc.tile_pool(name="persist", bufs=1) as persist,
            tc.tile_pool(name="xpool", bufs=2) as xpool,
            tc.tile_pool(name="kstage", bufs=3) as kstage,
            tc.tile_pool(name="vstage", bufs=3) as vstage,
            tc.tile_pool(name="pkpool", bufs=2) as pkpool,
            tc.tile_pool(name="pvpool", bufs=2) as pvpool,
            tc.tile_pool(name="epool", bufs=18) as epool,
            tc.tile_pool(name="cpool", bufs=3) as cpool,
            tc.tile_pool(name="rpool", bufs=4) as rpool,
            tc.tile_pool(name="opool", bufs=3) as opool,
            tc.tile_pool(name="pA", bufs=3, space="PSUM") as pA,
            tc.tile_pool(name="pB", bufs=2, space="PSUM") as pB,
            tc.tile_pool(name="pC", bufs=2, space="PSUM") as pC,
            tc.tile_pool(name="pD", bufs=1, space="PSUM") as pD,
        ):
            # ---- persistent SBUF tensors ----
            wq_sb = persist.tile([128, KO, HPC * HD], BF16, name="wq_sb")
            wk_sb = persist.tile([128, KO, HPC * HD], BF16, name="wk_sb")
            wv_sb = persist.tile([128, KO, HPC * HD], BF16, name="wv_sb")
            wo_sb = persist.tile([128, HPC, D], BF16, name="wo_sb")
            nc.sync.dma_start(wq_sb[:], wq_r[:])
            nc.sync.dma_start(wk_sb[:], wk_r[:])
            nc.sync.dma_start(wv_sb[:], wv_r[:])
            nc.sync.dma_start(wo_sb[:], wo_r[:])

            ident = persist.tile([128, 128], BF16, name="ident")
            make_identity(nc, ident)

            qT = [persist.tile([128, TOK], BF16, name=f"qT{h}") for h in range(HPC)]
            kTb = [persist.tile([128, TOK], BF16, name=f"kTb{h}") for h in range(HPC)]
            vsb = [persist.tile([128, NT, 132], BF16, name=f"vsb{h}") for h in range(HPC)]
            ctxT = [persist.tile([128, HPC, S], BF16, name=f"ctxT{b}") for b in range(B)]

            for h in range(HPC):
                nc.gpsimd.memset(vsb[h][:, :, 128:129], 1.0)

            # ---- phase 1: projections ----
            for tg in range(4):  # token groups of 512
                xt = xpool.tile([128, KO, 512], BF16, tag="xt")
                nc.sync.dma_start(xt[:], xT_r[:, :, ds(tg * 512, 512)])
                for h in range(HPC):
                    # qT[h][:, tg*512:+512]
                    pq = pA.tile([128, 512], F32, tag="pA")
                    for ko in range(KO):
                        nc.tensor.matmul(
                            pq[:], wq_sb[:, ko, ts(h, HD)], xt[:, ko, :],
                            start=(ko == 0), stop=(ko == KO - 1),
                        )
                    nc.scalar.copy(qT[h][:, ds(tg * 512, 512)], pq[:])

                    # kT
                    pk = pA.tile([128, 512], F32, tag="pA")
                    for ko in range(KO):
                        nc.tensor.matmul(
                            pk[:], wk_sb[:, ko, ts(h, HD)], xt[:, ko, :],
                            start=(ko == 0), stop=(ko == KO - 1),
                        )
                    nc.scalar.copy(kTb[h][:, ds(tg * 512, 512)], pk[:])
                    kst = kstage.tile([128, 512], F32, tag="kst")
                    nc.vector.tensor_copy(kst[:], pk[:])
                    nc.sync.dma_start(kT_out[h, :, ds(tg * 512, 512)], kst[:])

                    # v natural layout, 4 token tiles per group
                    for tt in range(4):
                        tau = tg * 4 + tt
                        pv_ = pB.tile([128, 128], F32, tag="pB")
                        for ko in range(KO):
                            nc.tensor.matmul(
                                pv_[:], xt[:, ko, ds(tt * 128, 128)],
                                wv_sb[:, ko, ts(h, HD)],
                                start=(ko == 0), stop=(ko == KO - 1),
                            )
                        nc.scalar.copy(vsb[h][:, tau, 0:128], pv_[:])
                        vst = vstage.tile([128, 128], F32, tag="vst")
                        nc.vector.tensor_copy(vst[:], pv_[:])
                        nc.sync.dma_start(v_out[h, ds(tau * 128, 128), :], vst[:])

            # ---- phase 2: attention + output projection ----
            for b in range(B):
                for h in range(HPC):
                    pk_sb = pkpool.tile([128, TP], BF16, tag="pk")
                    nc.sync.dma_start(pk_sb[:], pkT[b, h])
                    pv_sb = pvpool.tile([128, 8, 130], BF16, tag="pv")
                    nc.sync.dma_start(
                        pv_sb[:], pve[b, h].rearrange("(g p) e -> p g e", p=128)
                    )

                    exps = []
                    for g in range(16):
                        et = epool.tile([128, S], BF16, tag="exp")
                        exps.append(et)
                        if g < 8:
                            lhsT = pk_sb[:, ds(g * 128, 128)]
                            smin = 0
                        else:
                            gn = g - 8
                            smin = gn * 128
                            lhsT = kTb[h][:, ds(b * S + gn * 128, 128)]
                        c = smin
                        while c < S:
                            cw = min(512, S - c)
                            ps = pA.tile([128, 512], F32, tag="pA")
                            nc.tensor.matmul(
                                ps[:, :cw], lhsT, qT[h][:, ds(b * S + c, cw)],
                                start=True, stop=True,
                            )
                            nc.scalar.activation(
                                et[:, ds(c, cw)], ps[:, :cw],
                                mybir.ActivationFunctionType.Exp, scale=SCALE,
                            )
                            c += cw
                        if g >= 8:
                            # causal staircase on the diagonal block:
                            # keep where s_local - p >= 0, else 0
                            nc.gpsimd.affine_select(
                                out=et[:, ds(smin, 128)], in_=et[:, ds(smin, 128)],
                                pattern=[[1, 128]],
                                compare_op=mybir.AluOpType.is_ge,
                                fill=0.0, base=0, channel_multiplier=-1,
                            )

                    for si in range(8):
                        pc_ = pC.tile([128, 132], F32, tag="pC")
                        glist = list(range(8)) + [8 + gg for gg in range(si + 1)]
                        for j, g in enumerate(glist):
                            if g < 8:
                                rhs = pv_sb[:, g, 0:129]
                            else:
                                rhs = vsb[h][:, b * 8 + (g - 8), 0:129]
                            nc.tensor.matmul(
                                pc_[:, :129], exps[g][:, ds(si * 128, 128)], rhs,
                                start=(j == 0), stop=(j == len(glist) - 1),
                            )
                        rc = rpool.tile([128, 1], F32, tag="rc")
                        nc.vector.reciprocal(rc[:], pc_[:, 128:129])
                        cb = cpool.tile([128, 128], BF16, tag="cb")
                        nc.vector.tensor_scalar_mul(cb[:], pc_[:, 0:128], rc[:])
                        tp_ = pD.tile([128, 128], BF16, tag="pD")
                        nc.tensor.transpose(tp_[:], cb[:], ident[:])
                        nc.scalar.copy(ctxT[b][:, h, ds(si * 128, 128)], tp_[:])

                # output projection for this batch
                for mi in range(8):
                    for nj in range(4):
                        pw = pA.tile([128, 512], F32, tag="pA")
                        for ft in range(HPC):
                            nc.tensor.matmul(
                                pw[:], ctxT[b][:, ft, ds(mi * 128, 128)],
                                wo_sb[:, ft, ds(nj * 512, 512)],
                                start=(ft == 0), stop=(ft == HPC - 1),
                            )
                        ost = opool.tile([128, 512], F32, tag="ost")
                        nc.vector.tensor_copy(ost[:], pw[:])
                        nc.sync.dma_start(
                            partial[ds(b * S + mi * 128, 128), ds(nj * 512, 512)],
                            ost[:],
                        )

    nc.compile()
    return nc


def _get_nc():
    if "nc" not in _NC_CACHE:
        _NC_CACHE["nc"] = build_program()
    return _NC_CACHE["nc"]


def kernel(x, past_k, past_v, Wq, Wk, Wv, Wo):
    bf = ml_dtypes.bfloat16
    x = np.asarray(x, dtype=np.float32)
    past_k = np.asarray(past_k, dtype=np.float32)
    past_v = np.asarray(past_v, dtype=np.float32)
    Wq = np.asarray(Wq, dtype=np.float32)
    Wk = np.asarray(Wk, dtype=np.float32)
    Wv = np.asarray(Wv, dtype=np.float32)
    Wo = np.asarray(Wo, dtype=np.float32)

    x2 = x.reshape(TOK, D)
    xT_a = np.ascontiguousarray(x2.T).astype(bf)

    in_maps = []
    for c in range(8):
        hs = c * HPC
        r0, r1 = hs * HD, (hs + HPC) * HD
        wqT_a = np.ascontiguousarray(Wq[r0:r1, :].T).astype(bf)
        wkT_a = np.ascontiguousarray(Wk[r0:r1, :].T).astype(bf)
        wvT_a = np.ascontiguousarray(Wv[r0:r1, :].T).astype(bf)
        wo_a = np.ascontiguousarray(Wo[:, r0:r1].T).astype(bf)
        pkT_a = np.ascontiguousarray(
            past_k[:, hs:hs + HPC].transpose(0, 1, 3, 2)
        ).astype(bf)
        pve_a = np.zeros((B, HPC, TP, 130), dtype=bf)
        pve_a[..., :HD] = past_v[:, hs:hs + HPC].astype(bf)
        pve_a[..., HD] = 1.0
        in_maps.append(dict(
            xT=xT_a, wqT=wqT_a, wkT=wkT_a, wvT=wvT_a, wo=wo_a,
            pkT=pkT_a, pve=pve_a,
        ))

    nc = _get_nc()
    res = run_bass_kernel_spmd(nc, in_maps, core_ids=list(range(8)))

    out = np.zeros((TOK, D), dtype=np.float32)
    k_full = np.empty((B, H, TP + S, HD), dtype=np.float32)
    v_full = np.empty((B, H, TP + S, HD), dtype=np.float32)
    k_full[:, :, :TP] = past_k
    v_full[:, :, :TP] = past_v
    for c in range(8):
        r = res.results[c]
        out += r["partial"]
        kT_new = r["kT_out"]   # [HPC, HD, TOK]
        v_new = r["v_out"]     # [HPC, TOK, HD]
        for hh in range(HPC):
            h = c * HPC + hh
            kt = kT_new[hh].reshape(HD, B, S)
            k_full[:, h, TP:] = kt.transpose(1, 2, 0)
            v_full[:, h, TP:] = v_new[hh].reshape(B, S, HD)

    return out.reshape(B, S, D), k_full, v_full


# revision 11
# speedup vs baseline: 1.3465x; 1.3465x over previous
"""Fused attention block (QKV proj + KV-cache causal attention + output proj)
for Trainium2, tensor-parallel over heads across 8 NeuronCores.

Problem shapes (hardcoded): B=2, S=1024, T_past=1024, D=2048, H=16, hd=128.
Each core owns 2 heads. Inside each core:
  - qT/kT computed in [hd, token] layout (feature-major) via PE matmuls
  - v computed directly in [token, hd] layout
  - scores computed TRANSPOSED: scoresT[t, s] so the causal mask is a
    per-partition staircase (affine_select) and exp needs no max pass
    (scores ~ N(0,1), no overflow risk)
  - softmax denominator via a ones-column appended to v (col 128 of rhs)
  - ctx normalized per-partition (token on partitions), PE-transposed, then
    output projection accumulates the per-core partial in f32
Host side: shards/transposes inputs per core, sums the 8 partial outputs,
re-assembles k/v caches.
"""

import numpy as np
import ml_dtypes

import concourse.bass as bass
import concourse.bacc as bacc
import concourse.mybir as mybir
import concourse.tile as tile
from concourse.bass import ds, ts
from concourse.bass_utils import run_bass_kernel_spmd
from concourse.masks import make_identity

BF16 = mybir.dt.bfloat16
F32 = mybir.dt.float32

D = 2048
H = 16
HD = 128
B = 2
S = 1024
TP = 1024
HPC = 2               # heads per core
TOK = B * S           # 2048 tokens, b-major
KO = D // 128         # 16 contraction tiles
NT = TOK // 128       # 16 token tiles
SCALE = 1.0 / float(np.sqrt(HD))

_NC_CACHE = {}


def build_program():
    nc = bacc.Bacc("TRN2", target_bir_lowering=False, debug=False, num_devices=8)

    xT = nc.declare_dram_parameter("xT", [D, TOK], BF16, isOutput=False)
    wqT = nc.declare_dram_parameter("wqT", [D, HPC * HD], BF16, isOutput=False)
    wkT = nc.declare_dram_parameter("wkT", [D, HPC * HD], BF16, isOutput=False)
    wvT = nc.declare_dram_parameter("wvT", [D, HPC * HD], BF16, isOutput=False)
    wo = nc.declare_dram_parameter("wo", [HPC * HD, D], BF16, isOutput=False)
    pkT = nc.declare_dram_parameter("pkT", [B, HPC, HD, TP], BF16, isOutput=False)
    pve = nc.declare_dram_parameter("pve", [B, HPC, TP, 130], BF16, isOutput=False)

    kT_out = nc.declare_dram_parameter("kT_out", [HPC, HD, TOK], F32, isOutput=True)
    v_out = nc.declare_dram_parameter("v_out", [HPC, TOK, HD], F32, isOutput=True)
    partial = nc.declare_dram_parameter("partial", [TOK, D], BF16, isOutput=True)

    xT_r = xT.rearrange("(ko p) t -> p ko t", p=128)
    wq_r = wqT.rearrange("(ko p) m -> p ko m", p=128)
    wk_r = wkT.rearrange("(ko p) m -> p ko m", p=128)
    wv_r = wvT.rearrange("(ko p) m -> p ko m", p=128)
    wo_r = wo.rearrange("(ft p) n -> p ft n", p=128)

    with tile.TileContext(nc) as tc:
        with (
            tc.tile_pool(name="persist", bufs=1) as persist,
            tc.tile_pool(name="xpool", bufs=2) as xpool,
            tc.tile_pool(name="kstage", bufs=3) as kstage,
            tc.tile_pool(name="vstage", bufs=3) as vstage,
            tc.tile_pool(name="pkpool", bufs=2) as pkpool,
            tc.tile_pool(name="pvpool", bufs=2) as pvpool,
            tc.tile_pool(name="epool", bufs=18) as epool,
            tc.tile_pool(name="cpool", bufs=3) as cpool,
            tc.tile_pool(name="rpool", bufs=4) as rpool,
            tc.tile_pool(name="opool", bufs=3) as opool,
            tc.tile_pool(name="pA", bufs=3, space="PSUM") as pA,
            tc.tile_pool(name="pB", bufs=2, space="PSUM") as pB,
            tc.tile_pool(name="pC", bufs=2, space="PSUM") as pC,
            tc.tile_pool(name="pD", bufs=1, space="PSUM") as pD,
        ):
            # ---- persistent SBUF tensors ----
            wq_sb = persist.tile([128, KO, HPC * HD], BF16, name="wq_sb")
            wk_sb = persist.tile([128, KO, HPC * HD], BF16, name="wk_sb")
            wv_sb = persist.tile([128, KO, HPC * HD], BF16, name="wv_sb")
            wo_sb = persist.tile([128, HPC, D], BF16, name="wo_sb")
            nc.sync.dma_start(wq_sb[:], wq_r[:])
            # first x chunk right after wq so the q matmuls can start ASAP
            xt0 = xpool.tile([128, KO, 512], BF16, tag="xt")
            for kc in range(4):
                nc.sync.dma_start(
                    xt0[:, ds(kc * 4, 4), :], xT_r[:, ds(kc * 4, 4), ds(0, 512)]
                )
            nc.sync.dma_start(wk_sb[:], wk_r[:])
            nc.sync.dma_start(wv_sb[:], wv_r[:])

            ident = persist.tile([128, 128], BF16, name="ident")
            make_identity(nc, ident)

            qT = [persist.tile([128, TOK], BF16, name=f"qT{h}") for h in range(HPC)]
            kTb = [persist.tile([128, TOK], BF16, name=f"kTb{h}") for h in range(HPC)]
            vsb = [persist.tile([128, NT, 132], BF16, name=f"vsb{h}") for h in range(HPC)]
            ctxT = [persist.tile([128, HPC, S], BF16, name=f"ctxT{b}") for b in range(B)]

            for h in range(HPC):
                nc.gpsimd.memset(vsb[h][:, :, 128:129], 1.0)

            # ---- phase 1: projections ----
            for tg in range(4):  # token groups of 512
                if tg == 0:
                    xt = xt0
                else:
                    xt = xpool.tile([128, KO, 512], BF16, tag="xt")
                    # split the load so the first matmuls can start early
                    for kc in range(4):
                        nc.sync.dma_start(
                            xt[:, ds(kc * 4, 4), :],
                            xT_r[:, ds(kc * 4, 4), ds(tg * 512, 512)],
                        )
                for h in range(HPC):
                    # qT[h][:, tg*512:+512]
                    pq = pA.tile([128, 512], F32, tag="pA")
                    for ko in range(KO):
                        nc.tensor.matmul(
                            pq[:], wq_sb[:, ko, ts(h, HD)], xt[:, ko, :],
                            start=(ko == 0), stop=(ko == KO - 1),
                        )
                    nc.scalar.copy(qT[h][:, ds(tg * 512, 512)], pq[:])

                    # kT
                    pk = pA.tile([128, 512], F32, tag="pA")
                    for ko in range(KO):
                        nc.tensor.matmul(
                            pk[:], wk_sb[:, ko, ts(h, HD)], xt[:, ko, :],
                            start=(ko == 0), stop=(ko == KO - 1),
                        )
                    nc.scalar.copy(kTb[h][:, ds(tg * 512, 512)], pk[:])
                    kst = kstage.tile([128, 512], F32, tag="kst")
                    nc.vector.tensor_copy(kst[:], pk[:])
                    nc.sync.dma_start(kT_out[h, :, ds(tg * 512, 512)], kst[:])

                    # v natural layout, 4 token tiles per group
                    for tt in range(4):
                        tau = tg * 4 + tt
                        pv_ = pB.tile([128, 128], F32, tag="pB")
                        for ko in range(KO):
                            nc.tensor.matmul(
                                pv_[:], xt[:, ko, ds(tt * 128, 128)],
                                wv_sb[:, ko, ts(h, HD)],
                                start=(ko == 0), stop=(ko == KO - 1),
                            )
                        nc.vector.tensor_copy(vsb[h][:, tau, 0:128], pv_[:])
                        vst = vstage.tile([128, 128], F32, tag="vst")
                        nc.vector.tensor_copy(vst[:], pv_[:])
                        nc.sync.dma_start(v_out[h, ds(tau * 128, 128), :], vst[:])

            # wo only needed for the output projection; load after phase 1
            nc.sync.dma_start(wo_sb[:], wo_r[:])

            # ---- phase 2: attention + output projection ----
            for b in range(B):
                for h in range(HPC):
                    pk_sb = pkpool.tile([128, TP], BF16, tag="pk")
                    nc.sync.dma_start(pk_sb[:], pkT[b, h])
                    pv_sb = pvpool.tile([128, 8, 130], BF16, tag="pv")
                    nc.sync.dma_start(
                        pv_sb[:], pve[b, h].rearrange("(g p) e -> p g e", p=128)
                    )

                    exps = []
                    for g in range(16):
                        et = epool.tile([128, S], BF16, tag="exp")
                        exps.append(et)
                        if g < 8:
                            lhsT = pk_sb[:, ds(g * 128, 128)]
                            smin = 0
                        else:
                            gn = g - 8
                            smin = gn * 128
                            lhsT = kTb[h][:, ds(b * S + gn * 128, 128)]
                        c = smin
                        while c < S:
                            cw = min(512, S - c)
                            ps = pA.tile([128, 512], F32, tag="pA")
                            nc.tensor.matmul(
                                ps[:, :cw], lhsT, qT[h][:, ds(b * S + c, cw)],
                                start=True, stop=True,
                            )
                            nc.scalar.activation(
                                et[:, ds(c, cw)], ps[:, :cw],
                                mybir.ActivationFunctionType.Exp, scale=SCALE,
                            )
                            c += cw
                        if g >= 8:
                            # causal staircase on the diagonal block:
                            # keep where s_local - p >= 0, else 0
                            nc.gpsimd.affine_select(
                                out=et[:, ds(smin, 128)], in_=et[:, ds(smin, 128)],
                                pattern=[[1, 128]],
                                compare_op=mybir.AluOpType.is_ge,
                                fill=0.0, base=0, channel_multiplier=-1,
                            )

                    def finish_si(si, cb):
                        # PE transpose of the normalized ctx tile + Wo chunk;
                        # called one step late so the recip/normalize chain of
                        # step si is hidden behind PV matmuls of step si+1
                        tp_ = pD.tile([128, 128], BF16, tag="pD")
                        nc.tensor.transpose(tp_[:], cb[:], ident[:])
                        nc.vector.tensor_copy(ctxT[b][:, h, ds(si * 128, 128)], tp_[:])
                        if h == HPC - 1:
                            mi = si
                            for nj in range(4):
                                pw = pA.tile([128, 512], F32, tag="pA")
                                for ft in range(HPC):
                                    nc.tensor.matmul(
                                        pw[:], ctxT[b][:, ft, ds(mi * 128, 128)],
                                        wo_sb[:, ft, ds(nj * 512, 512)],
                                        start=(ft == 0), stop=(ft == HPC - 1),
                                    )
                                ost = opool.tile([128, 512], BF16, tag="ost")
                                if nj % 2 == 0:
                                    nc.vector.tensor_copy(ost[:], pw[:])
                                else:
                                    nc.scalar.copy(ost[:], pw[:])
                                nc.sync.dma_start(
                                    partial[ds(b * S + mi * 128, 128),
                                            ds(nj * 512, 512)],
                                    ost[:],
                                )

                    pend = None
                    for si in range(8):
                        pc_ = pC.tile([128, 132], F32, tag="pC")
                        glist = list(range(8)) + [8 + gg for gg in range(si + 1)]
                        for j, g in enumerate(glist):
                            if g < 8:
                                rhs = pv_sb[:, g, 0:129]
                            else:
                                rhs = vsb[h][:, b * 8 + (g - 8), 0:129]
                            nc.tensor.matmul(
                                pc_[:, :129], exps[g][:, ds(si * 128, 128)], rhs,
                                start=(j == 0), stop=(j == len(glist) - 1),
                            )
                        rc = rpool.tile([128, 1], F32, tag="rc")
                        nc.vector.reciprocal_approx_fast(rc[:], pc_[:, 128:129])
                        cb = cpool.tile([128, 128], BF16, tag="cb")
                        nc.scalar.activation(
                            cb[:], pc_[:, 0:128],
                            mybir.ActivationFunctionType.Copy,
                            bias=0.0, scale=rc[:],
                        )
                        if pend is not None:
                            finish_si(*pend)
                        pend = (si, cb)
                    finish_si(*pend)

    nc.compile()
    return nc


def _get_nc():
    if "nc" not in _NC_CACHE:
        _NC_CACHE["nc"] = build_program()
    return _NC_CACHE["nc"]


def kernel(x, past_k, past_v, Wq, Wk, Wv, Wo):
    bf = ml_dtypes.bfloat16
    x = np.asarray(x, dtype=np.float32)
    past_k = np.asarray(past_k, dtype=np.float32)
    past_v = np.asarray(past_v, dtype=np.float32)
    Wq = np.asarray(Wq, dtype=np.float32)
    Wk = np.asarray(Wk, dtype=np.float32)
    Wv = np.asarray(Wv, dtype=np.float32)
    Wo = np.asarray(Wo, dtype=np.float32)

    x2 = x.reshape(TOK, D)
    xT_a = np.ascontiguousarray(x2.T).astype(bf)

    in_maps = []
    for c in range(8):
        hs = c * HPC
        r0, r1 = hs * HD, (hs + HPC) * HD
        wqT_a = np.ascontiguousarray(Wq[r0:r1, :].T).astype(bf)
        wkT_a = np.ascontiguousarray(Wk[r0:r1, :].T).astype(bf)
        wvT_a = np.ascontiguousarray(Wv[r0:r1, :].T).astype(bf)
        wo_a = np.ascontiguousarray(Wo[:, r0:r1].T).astype(bf)
        pkT_a = np.ascontiguousarray(
            past_k[:, hs:hs + HPC].transpose(0, 1, 3, 2)
        ).astype(bf)
        pve_a = np.zeros((B, HPC, TP, 130), dtype=bf)
        pve_a[..., :HD] = past_v[:, hs:hs + HPC].astype(bf)
        pve_a[..., HD] = 1.0
        in_maps.append(dict(
            xT=xT_a, wqT=wqT_a, wkT=wkT_a, wvT=wvT_a, wo=wo_a,
            pkT=pkT_a, pve=pve_a,
        ))

    nc = _get_nc()
    res = run_bass_kernel_spmd(nc, in_maps, core_ids=list(range(8)))

    out = np.zeros((TOK, D), dtype=np.float32)
    k_full = np.empty((B, H, TP + S, HD), dtype=np.float32)
    v_full = np.empty((B, H, TP + S, HD), dtype=np.float32)
    k_full[:, :, :TP] = past_k
    v_full[:, :, :TP] = past_v
    for c in range(8):
        r = res.results[c]
        out += r["partial"].astype(np.float32)
        kT_new = r["kT_out"]   # [HPC, HD, TOK]
        v_new = r["v_out"]     # [HPC, TOK, HD]
        for hh in range(HPC):
            h = c * HPC + hh
            kt = kT_new[hh].reshape(HD, B, S)
            k_full[:, h, TP:] = kt.transpose(1, 2, 0)
            v_full[:, h, TP:] = v_new[hh].reshape(B, S, HD)

    return out.reshape(B, S, D), k_full, v_full
